# revision 56
# baseline (speedup 1.0000x reference)
"""Trainium2 Bass kernel for nn_DecoderCrossMSA (Swin-style shifted-window
cross-attention).

Strategy: data-parallel over batch (8 batches -> 8 cores). Host prepares, per
core, feature-major window-ordered activations (token axis permuted so every
8x8 shifted window is a contiguous 64-token run; roll folded into the
permutation). Device pipeline is software-pipelined over 32 window-pairs so
the tensor engine never waits on the softmax chain:

  iter t:  S(t) matmuls -> exp(t) [Act] -> pa(t)=exp*table [DVE]
           dd(t-1) row-sum matmul (block-diag ones) -> rd=1/dd, pn=pa*rd [DVE]
           AV(t-2) matmuls -> psum->SBUF copies [Act+DVE]
           + interleaved input/output projections of neighbouring blocks.

PSUM: st 4 banks (S scores), sm 2 banks (all projection psum + dd, 2-deep
rotation), av_sc/av_sh 1 bank each (compact AV output).
"""

import numpy as np
import ml_dtypes

EMB = 512
HEADS = 16
WS = 8
B = 8
HW = 64
N = HW * HW
EH = EMB // HEADS          # 32
WN = HW // WS              # 8
SHIFT = WS // 2            # 4
NW = WN * WN               # 64 windows
WT = WS * WS               # 64 tokens per window
NCORES = 8
NBLK = 8                   # token blocks per core (512 tokens each)
BLKT = N // NBLK           # 512
NPAIR = 32                 # window pairs per core
MASK_NEG = -30000.0

_bf16 = ml_dtypes.bfloat16
_f8 = ml_dtypes.float8_e4m3
SW1 = 256.0    # fp8 weight scale for W1 (incl 1/sqrt(32)), folded out in Act
SW2 = 64.0     # fp8 weight scale for W2


def _build_perm(shift):
    """perm[t] = token index n for window-ordered position t."""
    i, j, w1, w2 = np.meshgrid(
        np.arange(WN), np.arange(WN), np.arange(WS), np.arange(WS), indexing="ij"
    )
    r = (WS * i + w1 + shift) % HW
    c = (WS * j + w2 + shift) % HW
    return (r * HW + c).reshape(-1)


_PERM = _build_perm(SHIFT)
_OPERM = _build_perm(0)

# Reference splits EMB as (e H): head h lives on strided channels e*HEADS+h.
# Permute projection out-channels so head h is the contiguous block h*EH..:
_RHO = np.array([e * HEADS + h for h in range(HEADS) for e in range(EH)])


def _pair_tables(pos_emb):
    """4 pair-type tables [128, 16*64] bf16 of exp(T)^T, head-replicated."""
    idx = np.array([[x, y] for x in range(WS) for y in range(WS)])
    rel = idx[None, :, :] - idx[:, None, :] + WS - 1
    bias = pos_emb[rel[:, :, 0], rel[:, :, 1]].astype(np.float64)

    m = np.zeros((WT, WT), dtype=np.float64)
    s = WS * (WS // 2)
    m[-s:, :-s] = MASK_NEG
    m[:-s, -s:] = MASK_NEG
    r = WT // WS
    col = m.reshape(r, WS, r, WS).transpose(1, 0, 3, 2).reshape(WT, WT)

    t0 = bias
    t1 = bias + m          # row-masked  (i == 7)
    t2 = bias + col        # col-masked  (j == 7)
    t3 = bias + m + col    # corner

    def pair_tab(ta, tb):
        ea = np.exp(ta).T    # [k, q]
        eb = np.exp(tb).T
        stk = np.concatenate([ea, eb], axis=0)           # [128, 64]
        rep = np.tile(stk, (1, HEADS))                    # [128, 16*64]
        return rep.astype(_bf16)

    return np.stack([
        pair_tab(t0, t0),
        pair_tab(t0, t2),
        pair_tab(t1, t1),
        pair_tab(t1, t3),
    ])


def _pair_type(p):
    row = (p // 4) == WN - 1      # window-row i == 7
    colm = (p % 4) == 3           # second window j == 7
    return (2 if row else 0) + (1 if colm else 0)


def _build_bass():
    import concourse.mybir as mybir
    from concourse import bacc
    from concourse.tile import TileContext

    fp32 = mybir.dt.float32
    bf16 = mybir.dt.bfloat16
    AF = mybir.ActivationFunctionType
    ALU = mybir.AluOpType

    nc = bacc.Bacc()

    # ---- DRAM parameters (per-core) ----
    fp8 = mybir.dt.float8e4
    d_in = {}
    for name in ("cw", "sw"):
        d_in[name] = nc.declare_dram_parameter(name, [EMB, N], fp8, isOutput=False)
    for name in ("scw", "shw"):
        d_in[name] = nc.declare_dram_parameter(name, [EMB, N], bf16, isOutput=False)
    for name in ("w1q", "w2q"):
        d_in[name] = nc.declare_dram_parameter(name, [2, 128, 2 * EMB], fp8,
                                               isOutput=False)
    for name in ("wsct", "wsht", "wsot", "wshot"):
        d_in[name] = nc.declare_dram_parameter(name, [EMB, EMB], bf16, isOutput=False)
    for name in ("b1r", "b2r", "bsor", "bshor"):
        d_in[name] = nc.declare_dram_parameter(name, [128, 4], fp32, isOutput=False)
    d_in["ptab"] = nc.declare_dram_parameter(
        "ptab", [4, 128, HEADS * WT], bf16, isOutput=False
    )
    d_in["ones2"] = nc.declare_dram_parameter("ones2", [128, 128], bf16, isOutput=False)
    yso = nc.declare_dram_parameter("yso", [EMB, N], bf16, isOutput=True)
    ysho = nc.declare_dram_parameter("ysho", [EMB, N], bf16, isOutput=True)

    with TileContext(nc) as tc:
        with (
            tc.tile_pool(name="const", bufs=1) as cpool,
            tc.tile_pool(name="xg", bufs=3) as xgpool,
            tc.tile_pool(name="cs", bufs=3) as cspool,
            tc.tile_pool(name="v", bufs=8) as vpool,
            tc.tile_pool(name="smx", bufs=3) as smxp,
            tc.tile_pool(name="o", bufs=3) as opool,
            tc.tile_pool(name="y", bufs=6) as ypool,
            tc.tile_pool(name="stps", bufs=1, space="PSUM") as stps,
            tc.tile_pool(name="smps", bufs=2, space="PSUM") as smps,
            tc.tile_pool(name="avps", bufs=1, space="PSUM") as avps,
        ):

            # ---- constants into SBUF (QK-proj(0) deps first) ----
            wts = {}

            def load_w(name, eng):
                wts[name] = []
                for k in range(4):
                    t = cpool.tile([128, EMB], bf16, tag=f"{name}_{k}", name=f"{name}{k}")
                    eng.dma_start(t[:], d_in[name][k * 128:(k + 1) * 128, :])
                    wts[name].append(t)

            bias_t = {}

            def load_b(name, eng):
                t = cpool.tile([128, 4], fp32, tag=name, name=name)
                eng.dma_start(t[:], d_in[name][:])
                bias_t[name] = t

            wq = {}

            def load_wq(name):
                t = cpool.tile([128, 4 * EMB], fp8, tag=name, name=name)
                nc.sync.dma_start(
                    t[:].rearrange("p (s m) -> p s m", s=2),
                    d_in[name].rearrange("s p m -> p s m"),
                )
                wq[name] = [t[:, 0:2 * EMB], t[:, 2 * EMB:4 * EMB]]

            load_wq("w1q")

            # ---- pipeline state ----
            xg = {}        # blk -> {tensor: [4 k-chunk tiles]}
            cs = {}        # blk -> {"cw": [4], "sw": [4]}
            vt = {}        # pair g -> (vt_sc, vt_sh)
            pa_t = {}      # pair g -> pa tile
            pn_t = {}      # pair g -> pn tile
            osc_t = {}     # blk -> (osc, osh)

            def emit_xg_tensor(b, tname, eng=None, split=False):
                """One strided DMA loading all 4 k-chunks of tensor `tname`
                for block b into a single [128, 4*BLKT] tile (free = (chunk,
                token)). split=True loads chunk-pairs as two DMAs."""
                d = xg.setdefault(b, {})
                c0 = b * BLKT
                dt_ = fp8 if tname in ("cw", "sw") else bf16
                t = xgpool.tile([128, 4 * BLKT], dt_, tag=f"xga_{tname}",
                                name=f"xga{tname}")
                eng = eng or nc.sync
                srcv = d_in[tname][:, c0:c0 + BLKT].rearrange(
                    "(c p) t -> p c t", c=4
                )
                dstv = t[:].rearrange("p (c t) -> p c t", c=4)
                if split:
                    eng.dma_start(dstv[:, 0:2], srcv[:, 0:2])
                    eng.dma_start(dstv[:, 2:4], srcv[:, 2:4])
                else:
                    eng.dma_start(dstv, srcv)
                d[tname] = t

            def emit_xg_quarter(b, q, xq_eng=None):
                """Merged loads: one tensor per quarter."""
                emit_xg_tensor(b, ("cw", "sw", "scw", "shw")[q])

            def emit_qk_quarter(b, q):
                """Q/K projection output chunk m=q for block b (2 psum gens,
                fp8 DoubleRow: 2 accumulation steps of 256-wide contraction)."""
                d = cs.setdefault(b, {})
                for tname, wname, bname, sw in (("cw", "w1q", "b1r", SW1),
                                                ("sw", "w2q", "b2r", SW2)):
                    lst = d.setdefault(tname, [None] * 4)
                    ps = smps.tile([128, BLKT], fp32, tag="sm", name="psqk")
                    for s in range(2):
                        wv = wq[wname][s].rearrange(
                            "p (i m) -> p i m", i=2
                        )[:, :, q * 128:(q + 1) * 128]
                        xv = xg[b][tname][:, s * 2 * BLKT:(s + 1) * 2 * BLKT
                                          ].rearrange("p (i t) -> p i t", i=2)
                        nc.tensor.matmul(
                            ps[:], lhsT=wv, rhs=xv,
                            start=(s == 0), stop=(s == 1),
                            perf_mode=mybir.MatmulPerfMode.DoubleRow,
                        )
                    out = cspool.tile([128, BLKT], bf16, tag=f"cs_{tname}_{q}",
                                      name=f"cs{tname}{q}")
                    if tname == "cw":
                        nc.vector.tensor_scalar(
                            out[:], ps[:], 1.0 / sw,
                            bias_t[bname][:, q:q + 1],
                            ALU.mult, ALU.add,
                        )
                    else:
                        nc.scalar.activation(
                            out[:], ps[:], AF.Identity,
                            bias=bias_t[bname][:, q:q + 1], scale=1.0 / sw,
                        )
                    lst[q] = out

            def emit_v_quarter(b, q):
                """V projections (token-major) for pair g = 4b+q."""
                g = 4 * b + q
                t0 = q * 128
                pair = []
                for tname, wname, vtag, eng in (("scw", "wsct", "vsc", "v"),
                                                ("shw", "wsht", "vsh", "a")):
                    ps = smps.tile([128, EMB], fp32, tag="sm", name="psv")
                    for k in range(4):
                        nc.tensor.matmul(
                            ps[:],
                            lhsT=xg[b][tname][:, k * BLKT + t0:k * BLKT + t0 + 128],
                            rhs=wts[wname][k][:],
                            start=(k == 0), stop=(k == 3),
                        )
                    out = vpool.tile([128, EMB], bf16, tag=vtag, name=vtag)
                    if eng == "v":
                        nc.vector.tensor_copy(out[:], ps[:])
                    else:
                        nc.scalar.activation(out[:], ps[:], AF.Copy)
                    pair.append(out)
                vt[g] = pair

            def emit_S(g):
                blk, p = g // 4, g % 4
                t0 = p * 128
                cT, sT = cs[blk]["cw"], cs[blk]["sw"]
                st = stps.tile([128, 4 * 512], fp32, tag="st", name="st")
                for h in range(HEADS):
                    m, r = h // 4, (h % 4) * 32
                    s0 = (h % 4) * 512 + (h // 4) * WT
                    for wi in range(2):
                        o0 = t0 + wi * WT
                        nc.tensor.matmul(
                            st[wi * WT:(wi + 1) * WT, s0:s0 + WT],
                            lhsT=sT[m][r:r + 32, o0:o0 + WT],
                            rhs=cT[m][r:r + 32, o0:o0 + WT],
                            start=True, stop=True,
                            tile_position=(r, wi * WT),
                        )
                st_v = st[:].rearrange("p (b s q) -> p b s q", b=4, s=8, q=WT)[:, :, 0:4, :]
                pe = smxp.tile([128, HEADS * WT], bf16, tag="pe", name="pe")
                pe_v = pe[:].rearrange("p (b s q) -> p b s q", b=4, s=4, q=WT)
                nc.scalar.activation(pe_v, st_v, AF.Exp)
                pa = smxp.tile([128, HEADS * WT], bf16, tag="pa", name="pa")
                nc.vector.tensor_tensor(
                    pa[:], pe[:], ptab_t[_pair_type(g)][:], ALU.mult
                )
                pa_t[g] = pa

            def emit_rowsum(g):
                """dd = per-window column sums of pa(g); pn = pa * (1/dd)."""
                pa = pa_t.pop(g)
                rd = smxp.tile([128, HEADS * WT], bf16, tag="rd", name="rd")
                for half in range(2):
                    fs = slice(half * 512, (half + 1) * 512)
                    dd = smps.tile([128, 512], fp32, tag="sm", name="dd")
                    nc.tensor.matmul(
                        dd[:], lhsT=ones2_t[:], rhs=pa[:, fs],
                        start=True, stop=True,
                    )
                    with nc.allow_low_precision(reason="bf16 softmax denom"):
                        nc.vector.reciprocal(rd[:, fs], dd[:])
                pn = smxp.tile([128, HEADS * WT], bf16, tag="pn", name="pn", bufs=4)
                nc.vector.tensor_tensor(pn[:], pa[:], rd[:], ALU.mult)
                pn_t[g] = pn

            def emit_AV(g):
                """AV matmuls for pair g into one [128, 1024] PSUM tile:
                free = wi*512 + tensor*256 + m*64 + q. Bank = wi (PE row-group
                -> own bank, the HW wiring rule); sc/sh and h vs h+4 share a
                quadrant so their result streams serialize safely."""
                blk, p = g // 4, g % 4
                t0 = p * 128
                pn = pn_t.pop(g)
                vsc, vsh = vt.pop(g)
                av = avps.tile([128, 1024], fp32, tag="av", name="av")
                for h in range(HEADS):
                    m, r = h // 4, (h % 4) * 32
                    ps0 = (h % 4) * 256 + (h // 4) * WT
                    for wi in range(2):
                        sl = slice(wi * WT, (wi + 1) * WT)
                        for ti, vtile in ((0, vsc), (1, vsh)):
                            f0 = wi * 512 + ti * 256 + m * WT
                            nc.tensor.matmul(
                                av[r:r + 32, f0:f0 + WT],
                                lhsT=vtile[sl, h * 32:(h + 1) * 32],
                                rhs=pn[sl, ps0:ps0 + WT],
                                start=True, stop=True,
                                tile_position=(wi * WT, r),
                            )
                if p == 0:
                    osc = opool.tile([128, 4 * BLKT], bf16, tag="osc", name="osc")
                    osh = opool.tile([128, 4 * BLKT], bf16, tag="osh", name="osh")
                    osc_t[blk] = (osc, osh)
                osc, osh = osc_t[blk]
                # scatter [128, (2 wi, 2 tensor, 4 m, 64 q)] into O tiles
                srcv = av[:].rearrange("p (w t m q) -> p t m w q", w=2, t=2,
                                       m=4, q=WT)
                for ti, o_t, eng in ((0, osc, "a"), (1, osh, "v")):
                    dstv = o_t[:].rearrange("p (m t) -> p m t", m=4)
                    dst = dstv[:, :, t0:t0 + 128].rearrange(
                        "p m (w q) -> p m w q", w=2
                    )
                    if eng == "a":
                        nc.scalar.activation(dst, srcv[:, ti], AF.Copy)
                    else:
                        nc.vector.tensor_copy(dst, srcv[:, ti])

            def emit_OP_half(b, half):
                osc, osh = osc_t[b]
                o_t, wname, bname, y_h = (
                    (osc, "wsot", "bsor", yso) if half == 0
                    else (osh, "wshot", "bshor", ysho)
                )
                c0 = b * BLKT
                last = (b == NBLK - 1)
                for mo in range(4):
                    ps = smps.tile([128, BLKT], fp32, tag="sm", name="psop")
                    for k in range(4):
                        nc.tensor.matmul(
                            ps[:],
                            lhsT=wts[wname][k][:, mo * 128:(mo + 1) * 128],
                            rhs=o_t[:, k * BLKT:(k + 1) * BLKT],
                            start=(k == 0), stop=(k == 3),
                        )
                    y_sb = ypool.tile([128, BLKT], bf16, tag="y", name="ysb")
                    if mo % 2 == 1:
                        with nc.allow_low_precision(reason="bf16 output"):
                            nc.vector.tensor_scalar_add(
                                y_sb[:], ps[:], bias_t[bname][:, mo:mo + 1],
                            )
                    else:
                        nc.scalar.activation(
                            y_sb[:], ps[:], AF.Identity,
                            bias=bias_t[bname][:, mo:mo + 1],
                        )
                    eng = nc.sync if last else nc.gpsimd
                    eng.dma_start(
                        y_h[mo * 128:(mo + 1) * 128, c0:c0 + BLKT], y_sb[:]
                    )
                if half == 1:
                    del osc_t[b]

            def emit_dummies(n):
                """Keep PE busy (and its p-state ramp alive) during prologue
                DMA waits: harmless matmuls on the first-loaded weight tile
                into the not-yet-used av psum slot."""
                wt = wq["w1q"][0]
                for i in range(n):
                    dmy = avps.tile([128, 1024], fp32, tag="av", name="dmy")
                    nc.tensor.matmul(
                        dmy[:, 0:512], lhsT=wt[:, 0:128], rhs=wt[:, 0:512],
                        start=True, stop=True,
                    )

            # ---- prologue: inputs + projections for blocks 0 and 1 ----
            emit_dummies(0)
            load_wq("w2q")
            load_b("b1r", nc.sync)
            load_b("b2r", nc.sync)
            emit_xg_tensor(0, "cw", split=True)
            emit_xg_tensor(0, "sw", eng=nc.gpsimd, split=True)
            for q in range(4):
                emit_qk_quarter(0, q)
            load_w("wsct", nc.sync)
            emit_xg_tensor(0, "scw", split=True)
            load_w("wsht", nc.sync)
            emit_xg_tensor(0, "shw", eng=nc.gpsimd, split=True)
            emit_dummies(8)
            ones2_t = cpool.tile([128, 128], bf16, tag="ones2", name="ones2")
            nc.gpsimd.dma_start(ones2_t[:], d_in["ones2"][:])
            ptab_t = []
            for i in range(4):
                t = cpool.tile([128, HEADS * WT], bf16, tag=f"ptab{i}", name=f"ptab{i}")
                nc.gpsimd.dma_start(t[:], d_in["ptab"][i])
                ptab_t.append(t)
            load_w("wsot", nc.gpsimd)
            load_w("wshot", nc.gpsimd)
            load_b("bsor", nc.gpsimd)
            load_b("bshor", nc.gpsimd)
            for q in range(4):
                emit_v_quarter(0, q)
            for tname in ("cw", "sw", "scw", "shw"):
                emit_xg_tensor(1, tname)

            # ---- software-pipelined main loop ----
            for t in range(36):
                bn_x = t // 4 + 2      # block whose inputs we DMA
                bn_p = t // 4 + 1      # block whose projections we compute
                q = t % 4
                if 1 <= t <= 32:
                    emit_rowsum(t - 1)
                if t < 32:
                    emit_S(t)
                if bn_x < NBLK:
                    emit_xg_quarter(bn_x, q)
                if bn_p < NBLK:
                    emit_qk_quarter(bn_p, q)
                    emit_v_quarter(bn_p, q)
                if 2 <= t <= 33:
                    emit_AV(t - 2)
                tb = t - 6
                if tb >= 0 and tb % 4 in (0, 1) and tb // 4 < NBLK - 2:
                    emit_OP_half(tb // 4, tb % 4)
                if t == 30:
                    emit_OP_half(6, 0)
                if t == 32:
                    emit_OP_half(6, 1)
                if t == 33:
                    emit_OP_half(7, 0)
                    emit_OP_half(7, 1)

    nc.compile()
    return nc


_NC_CACHE = {}
LAST_RESULT = None


def make_in_maps(content, style, scale, shift, W1, b1, W2, b2, Wsc, bsc,
                 Wsh, bsh, Wso, bso, Wsho, bsho, pos_emb):
    inv = 1.0 / np.sqrt(EMB / HEADS)
    f32 = np.float32

    def dr_pack(wt):
        # [512, 512] fp32 -> [2 steps, 128, (2 interleave, 512 out)] fp8:
        # step s partition k pairs channels s*256+k (i=0) and s*256+128+k.
        w = wt.reshape(2, 2, 128, EMB)                 # [s, i, k, m]
        return np.ascontiguousarray(w.transpose(0, 2, 1, 3).reshape(
            2, 128, 2 * EMB)).astype(_f8)

    w1q = dr_pack(np.asarray(W1, f32)[_RHO].T * inv * SW1)
    w2q = dr_pack(np.asarray(W2, f32)[_RHO].T * SW2)
    wsct = np.asarray(Wsc, f32)[_RHO].T.astype(_bf16)
    wsht = np.asarray(Wsh, f32)[_RHO].T.astype(_bf16)
    wsot = np.asarray(Wso, f32).T.astype(_bf16)
    wshot = np.asarray(Wsho, f32).T.astype(_bf16)
    b1r = (np.asarray(b1, f32)[_RHO] * inv).reshape(4, 128).T.copy()
    b2r = np.asarray(b2, f32)[_RHO].reshape(4, 128).T.copy()
    bso2 = np.asarray(Wso, f32) @ np.asarray(bsc, f32)[_RHO] + np.asarray(bso, f32)
    bsho2 = (np.asarray(Wsho, f32) @ np.asarray(bsh, f32)[_RHO]
             + np.asarray(bsho, f32))
    bsor = bso2.reshape(4, 128).T.copy()
    bshor = bsho2.reshape(4, 128).T.copy()
    ptab = _pair_tables(np.asarray(pos_emb, f32))
    ones2 = np.zeros((128, 128), dtype=_bf16)
    ones2[:64, :64] = 1
    ones2[64:, 64:] = 1

    common = dict(
        w1q=w1q, w2q=w2q, wsct=wsct, wsht=wsht, wsot=wsot, wshot=wshot,
        b1r=b1r, b2r=b2r, bsor=bsor, bshor=bshor, ptab=ptab, ones2=ones2,
    )
    in_maps = []
    for b in range(NCORES):
        m = dict(common)
        for name, full in (("cw", content), ("sw", style)):
            x = np.asarray(full[b], f32)[_PERM]           # [N, EMB] window order
            m[name] = np.ascontiguousarray(x.T).astype(_f8)
        for name, full in (("scw", scale), ("shw", shift)):
            x = np.asarray(full[b], f32)[_PERM]
            m[name] = np.ascontiguousarray(x.T).astype(_bf16)
        in_maps.append(m)
    return in_maps


def kernel(**inputs):
    global LAST_RESULT
    from concourse.bass_utils import run_bass_kernel_spmd

    in_maps = make_in_maps(**inputs)

    if "nc" not in _NC_CACHE:
        _NC_CACHE["nc"] = _build_bass()
    res = run_bass_kernel_spmd(_NC_CACHE["nc"], in_maps, list(range(NCORES)))
    LAST_RESULT = res

    out_sc = np.empty((B, N, EMB), np.float32)
    out_sh = np.empty((B, N, EMB), np.float32)
    for b in range(NCORES):
        out_sc[b][_OPERM] = res.results[b]["yso"].T
        out_sh[b][_OPERM] = res.results[b]["ysho"].T
    return out_sc, out_sh
